# revision 36
# baseline (speedup 1.0000x reference)
# Grouped GEMM (MoE) kernel for Trainium2, 8 NeuronCores.
#
# Sharding: tensor-parallel over out_features (column parallel). Each core
# computes ALL 4096 tokens against its own 416-column slice of every
# expert's weight. No collectives; host concatenates per-core outputs
# along the feature axis. This is perfectly load balanced across cores
# regardless of the (uneven) per-expert token counts, and the program is
# identical on every core (SPMD) -- only the weight *values* differ.
#
# Dtype: inputs are cast to bf16 on host (PE runs bf16 at 4x the fp32
# rate; tolerance 2e-2 vs ~2.5e-3 bf16 error). Output is stored bf16 and
# upcast on host, halving store traffic.
#
# The kernel is INPUT-DMA-BOUND: ~42MB/core over 16 DMA engines that
# peak ~22GB/s each. Measured engine efficiency is ~84% with the naive
# layout because HWDGE emits one descriptor per (partition, k-tile) line
# (832-2048B) and pays ~7ns per descriptor. So the host packs every
# transfer as ONE contiguous per-partition run (8-20KB descriptors):
#   x: per 512-token chunk, all 20 k-tiles contiguous [p | k | t]
#   w: per 10-k-tile half slice, [p | k | c] (already contiguous)
#   x tails (for the tail waves): packed separately the same way
# SBUF tiles are allocated FLAT and rearranged into [p, k, *] views for
# the matmuls, so the DMA sees identical src/dst patterns.
#
# PE structure:
#  * psum[tok, col] += x_tile[k, tok].T @ w_tile[k, col], k accumulated
#    over 20 k-tiles per 128-token m-tile.
#  * Partial (tail) m-tiles are NOT processed as full 416-cycle m-tiles.
#    Tails are split into <=32-token subtiles and packed 4-at-a-time into
#    the PE's 32-column groups via tile_position=(0,32j): the 4 streams
#    run concurrently, so a wave of 4 tails costs ~1 m-tile instead of 4.
#  * HAM warm-up: ~12 dummy matmuls on a memset tile run during the DMA
#    ramp so the PE clock gate is at 8/8 when the real matmuls start.
#  * Expert order is searched (all permutations, simulated) to keep the
#    cumulative DMA demand behind the cumulative PE supply.
#
# All input DMAs issue from SP in w-half0, chunk0, w-half1, chunk1 order
# per expert: the DMA engines serve queues at similar rates regardless
# of backlog, so splitting w/x across queues skews their delivery ratio
# (w on GpSimd's queue measured ~10us slower; x on ACT 41us slower).
# Stores issue from ACT so their waits never stall input issue; the tiny
# tail-x loads issue from GpSimd.

import os

import numpy as np

NUM_TOKENS = 4096
IN_FEATURES = 2560
OUT_FEATURES = 3328
GROUPS = 8
N_CORES = 8
COLS = OUT_FEATURES // N_CORES  # 416
P = 128
K_TILES = IN_FEATURES // P  # 20
K_PIECES = (5, 5, 10)  # w pieces: the small leading pieces unblock the
# first matmuls ~2us earlier during the ramp (0.53MB vs 1.06MB wait)
CHUNK = 512  # tokens per x chunk; chunks release their pool slot at
# their own last m-tile (not expert end) so the 6-deep rotation gives
# ~3 experts of DMA lookahead and SP's in-order issue rarely blocks
SUB = 32  # tail subtile height (PE column-group width)
W_BUFS = 4  # weight pool depth (experts in flight per k-half tag)
X_BUFS = 6  # x pool depth (512-token chunks in flight)

LAST_EXEC_TIME_NS = None
LAST_TRACE = None
LAST_RESULT = None

_COMPILED = {}
_LAYOUTS = {}


def _plan(sizes):
    """Pick expert order + tail-wave packing.

    A tail wave is <=4 subtiles (each <=32 tokens) matmul'd concurrently
    in the PE's four 32-column groups. A wave reads the w and tail-x SBUF
    tiles of its member experts, so it must be emitted before those
    pool slots are reallocated (w rotates W_BUFS deep, tail-x 3 deep)
    => a member of age a (experts processed since it) needs a <= 2.

    Search all expert permutations; minimize wave count, prefer a
    tail-less final expert, then maximize the worst prefix slack of
    (PE work supplied) - (DMA bytes demanded).
    """
    import itertools

    max_age = 2
    nz = [g for g in range(GROUPS) if int(sizes[g]) > 0]
    full = {g: int(sizes[g]) // P for g in nz}
    subs = {}
    for g in nz:
        t = int(sizes[g]) % P
        s, off = [], full[g] * P
        while t > 0:
            m = min(SUB, t)
            s.append((off, m))
            off += m
            t -= m
        subs[g] = s

    mm_us = K_TILES * COLS / 2.4e3  # warm full-tile cost, us
    wave_us = K_TILES * (COLS / 2.4 + 12) / 1e3
    bw = 0.35  # GB/s -> MB/us
    wslice_mb = IN_FEATURES * COLS * 2 / 1e6

    def simulate(order):
        pending = []  # (pos, g, off, mt)
        waves_after = [[] for _ in order]
        pe = dma = 0.0
        min_slack = 1e9
        nwaves = 0
        for pos, g in enumerate(order):
            dma += wslice_mb + int(sizes[g]) * IN_FEATURES * 2 / 1e6
            pe += full[g] * mm_us
            pending += [(pos, g, o, m) for (o, m) in subs[g]]
            last = pos == len(order) - 1
            while pending and (
                last or pos - pending[0][0] >= max_age or len(pending) > 4
            ):
                wave = pending[:4]
                pending = pending[4:]
                waves_after[pos].append([(g2, o, m) for (_, g2, o, m) in wave])
                nwaves += 1
                pe += wave_us
            min_slack = min(min_slack, pe - dma / bw)
        return nwaves, min_slack, waves_after

    best = None
    for order in itertools.permutations(nz):
        nwaves, slack, waves_after = simulate(order)
        ends_with_tail = 1 if subs[order[-1]] else 0
        key = (nwaves, ends_with_tail, -slack)
        if best is None or key < best[0]:
            best = (key, order, waves_after)
    return best[1], best[2]


def _layout(sizes):
    """Flat element layout of the packed x buffer (per partition line).

    chunks[g] = [(flat_off, cbase, clen), ...]
    tails[g] = (flat_off, tail_base, tail_len)
    """
    if sizes in _LAYOUTS:
        return _LAYOUTS[sizes]
    order, waves_after = _plan(sizes)
    offs = [0]
    for s in sizes:
        offs.append(offs[-1] + int(s))
    pos = 0
    chunks = {}
    tails = {}
    for g in order:
        seg = int(sizes[g])
        li = []
        # first expert gets a one-m-tile starter chunk so the first real
        # matmul only waits for 1.7MB (w-half0 + 128 tokens) of DMA
        # instead of 3.7MB -- cuts ~5us off the pipeline ramp
        # the first expert's chunks grow progressively [128,128,256,512..]
        # so the DMA supply curve tracks the PE's m-tile demand curve
        # during the pipeline ramp instead of stalling m-tiles 1-4 behind
        # one 2.6MB chunk
        cbase = 0
        if g == order[0]:
            for clen in (P, P, 2 * P):
                if cbase + clen > seg:
                    break
                li.append((pos, cbase, clen))
                pos += K_TILES * clen
                cbase += clen
        while cbase < seg:
            clen = min(CHUNK, seg - cbase)
            li.append((pos, cbase, clen))
            pos += K_TILES * clen
            cbase += clen
        chunks[g] = li
        tl = seg % P
        if tl:
            tails[g] = (pos, (seg // P) * P, tl)
            pos += K_TILES * tl
    _LAYOUTS[sizes] = (order, waves_after, offs, chunks, tails, pos)
    return _LAYOUTS[sizes]


def _build(sizes, dt_name, out_dt_name, reps=1):
    import concourse.bass as bass
    import concourse.mybir as mybir
    import concourse.tile as tile

    dt_in = getattr(mybir.dt, dt_name)
    dt_out = getattr(mybir.dt, out_dt_name)
    f32 = mybir.dt.float32

    order, waves_after, offs, chunks, tails, x_total = _layout(sizes)

    nc = bass.Bass()
    xt_d = nc.dram_tensor("xt", [P, x_total], dt_in, kind="ExternalInput")
    wt_d = nc.dram_tensor(
        "wt", [GROUPS, P, K_TILES * COLS], dt_in, kind="ExternalInput"
    )
    out_d = nc.dram_tensor("out", [NUM_TOKENS, COLS], dt_out, kind="ExternalOutput")
    # tail-wave staging: one [128, COLS] block per wave, scattered to the
    # right output rows on the host (one big store beats 4 tiny ones)
    n_waves_max = (GROUPS * 4 + 3) // 4
    tout_d = nc.dram_tensor(
        "tout", [n_waves_max * P, COLS], dt_out, kind="ExternalOutput"
    )

    # Tile's default kernel tail is: drain -> barrier -> clear all tile
    # semaphores -> barrier. The drain already guarantees every DMA
    # completed; the sem clears only matter for re-executing the same
    # loaded NEFF, which the runtime re-inits anyway. Keep drain + one
    # barrier, skip the clears.
    from concourse.vector_clock import ScopedClock

    if not hasattr(tile.TileContext, "_orig_drain_and_barrier"):
        tile.TileContext._orig_drain_and_barrier = tile.TileContext._drain_and_barrier

    def _short_drain_and_barrier(self, tick_clock, wait_clock):
        if os.environ.get("GG_FULL_TAIL", "0") == "1":
            return tile.TileContext._orig_drain_and_barrier(
                self, tick_clock, wait_clock
            )
        drain_inst = self.nc.sync.drain()
        wait_clock.add_sem_waits(
            drain_inst.ins, ScopedClock({None: tick_clock.global_clock})
        )
        self.nc.all_engine_barrier()
        popped = self.nc._tile_sem_poison_stack.pop()
        assert popped is self._sem_poison

    tile.TileContext._drain_and_barrier = _short_drain_and_barrier

    with tile.TileContext(nc) as tc:
        with (
            tc.tile_pool(name="wp", bufs=W_BUFS) as wp,
            tc.tile_pool(name="xp", bufs=X_BUFS) as xp,
            tc.tile_pool(name="xtp", bufs=3) as xtp,
            tc.tile_pool(name="pp", bufs=6, space="PSUM") as pp,
            tc.tile_pool(name="op", bufs=3) as op,
        ):
            def body():
                _emit_body(
                    nc, wp, xp, xtp, pp, op, sizes, dt_in, dt_out, f32,
                    xt_d, wt_d, out_d, tout_d,
                    order, waves_after, offs, chunks, tails,
                )

            if reps > 1:
                with tc.For_i(0, reps, 1):
                    body()
            else:
                body()

    _split_waits(nc, mybir)
    nc.finalize()
    return nc


def _emit_body(nc, wp, xp, xtp, pp, op, sizes, dt_in, dt_out, f32,
               xt_d, wt_d, out_d, tout_d, order, waves_after, offs, chunks, tails):
    tw = (max(1, max(int(s) % P for s in sizes)) + 7) // 8 * 8

    # --- HAM warm-up: ~6us of dummy matmuls with no DMA deps. They run
    # during the DMA ramp (PE would be idle anyway) and flip the PE clock
    # gate to 8/8 before the first real matmul arrives.
    wrm = op.tile([P, SUB + COLS], dt_in, tag="wrm", bufs=1, name="wrm")
    nc.vector.memset(wrm[:, :], 0)
    for i in range(16):
        wps = pp.tile([P, COLS], f32, tag="wv", bufs=2, name=f"wps_{i}")
        nc.tensor.matmul(
            wps[:SUB, :P],
            wrm[:, :SUB],
            wrm[:, SUB : SUB + P],
            start=True,
            stop=True,
        )

    kb = [0]
    for n in K_PIECES:
        kb.append(kb[-1] + n)  # piece q covers k in [kb[q], kb[q+1])

    wtiles = {}  # g -> [k-view per half]
    xtiles = {}  # g -> [k-view per chunk]
    xtails = {}  # g -> (k-view, tail_base)

    wave_idx = [0]
    wave_map = []  # (wave_slot, j, g, soff, mt) for the host scatter

    def emit_wave(wave):
        ps = pp.tile([P, COLS], f32, tag="wv", bufs=2, name="ps_wave")
        for k in range(K_TILES):
            q = next(i for i in range(len(K_PIECES)) if kb[i + 1] > k)
            r = k - kb[q]
            for j, (g, soff, mt) in enumerate(wave):
                xtv, tb = xtails[g]
                sc = soff - tb
                nc.tensor.matmul(
                    ps[SUB * j : SUB * j + mt, :],
                    xtv[:, k, sc : sc + mt],
                    wtiles[g][q][:, r, :],
                    start=(k == 0),
                    stop=(k == K_TILES - 1),
                    tile_position=(0, SUB * j),
                )
        ob = op.tile([P, COLS], dt_out, tag="o", name="ob_wave")
        nc.vector.tensor_copy(ob[:, :], ps[:, :])
        wi = wave_idx[0]
        wave_idx[0] += 1
        nc.scalar.dma_start(tout_d[wi * P : (wi + 1) * P, :], ob[:, :])
        for j, (g, soff, mt) in enumerate(wave):
            wave_map.append((wi, j, g, soff, mt))

    for pos, g in enumerate(order):
        seg = int(sizes[g])
        off = offs[g]
        wtiles[g] = []
        xtiles[g] = []

        def emit_x(ci):
            fo, cbase, clen = chunks[g][ci]
            xt = xp.tile([P, K_TILES * clen], dt_in, tag="x", name=f"x_{g}_{ci}")
            nc.sync.dma_start(xt[:, :], xt_d[:, fo : fo + K_TILES * clen])
            xtiles[g].append(xt[:, :].rearrange("p (k t) -> p k t", k=K_TILES))

        def emit_w(q):
            kn = K_PIECES[q]
            wt = wp.tile([P, kn * COLS], dt_in, tag=f"w{q}", name=f"w_{g}_{q}")
            nc.sync.dma_start(
                wt[:, :], wt_d[g, :, kb[q] * COLS : kb[q + 1] * COLS]
            )
            wtiles[g].append(wt[:, :].rearrange("p (k c) -> p k c", k=kn))

        emit_w(0)
        emit_x(0)
        for q in range(1, len(K_PIECES)):
            emit_w(q)
        for ci in range(1, len(chunks[g])):
            emit_x(ci)
        if g in tails:
            fo, tb, tl = tails[g]
            xtt = xtp.tile([P, K_TILES * tw], dt_in, tag="xt", name=f"xtail_{g}")
            nc.gpsimd.dma_start(
                xtt[:, : K_TILES * tl], xt_d[:, fo : fo + K_TILES * tl]
            )
            xtails[g] = (
                xtt[:, : K_TILES * tl].rearrange("p (k t) -> p k t", k=K_TILES),
                tb,
            )

        # on the last expert, flush the remaining waves BEFORE its m-loop:
        # their stores then overlap its compute instead of serializing at
        # the very end of the kernel (members are from recent experts, so
        # their tiles are still live)
        if pos == len(order) - 1:
            for wave in waves_after[pos]:
                emit_wave(wave)

        # full 128-token m-tiles only; tails go to the packed waves
        for ci in range(len(chunks[g])):
            fo, cbase, clen = chunks[g][ci]
            n_full = clen // P
            for m in range(n_full):
                ps = pp.tile([P, COLS], f32, tag="ps", name=f"ps_{g}_{ci}_{m}")
                for k in range(K_TILES):
                    q = next(i for i in range(len(K_PIECES)) if kb[i + 1] > k)
                    r = k - kb[q]
                    nc.tensor.matmul(
                        ps[:, :],
                        xtiles[g][ci][:, k, m * P : (m + 1) * P],
                        wtiles[g][q][:, r, :],
                        start=(k == 0),
                        stop=(k == K_TILES - 1),
                    )
                ob = op.tile([P, COLS], dt_out, tag="o", name=f"ob_{g}_{ci}_{m}")
                nc.vector.tensor_copy(ob[:, :], ps[:, :])
                r0 = off + cbase + m * P
                nc.scalar.dma_start(out_d[r0 : r0 + P, :], ob[:, :])

        if pos < len(order) - 1:
            for wave in waves_after[pos]:
                emit_wave(wave)

    nc._gg_wave_map = wave_map


def _split_waits(nc, mybir):
    """This container's walrus build allows at most ONE sync wait per
    instruction ('Too many sync wait commands' otherwise). Split any
    instruction carrying N>1 waits into N-1 same-engine NoOps (one wait
    each) followed by the original instruction with the last wait. Engine
    sequencers execute in order, so semantics are preserved."""
    counter = [0]
    for blk in nc.m.functions[0].blocks:
        insts = blk.instructions
        out = []
        changed = False
        for inst in insts:
            si = inst.sync_info
            if si is not None and len(si.on_wait) > 1:
                waits = list(si.on_wait)
                for w in waits[:-1]:
                    counter[0] += 1
                    nop = mybir.InstNoOp(name=f"I-nopw-{counter[0]}")
                    nop.engine = inst.engine
                    nop.sync_info = mybir.SyncInfo(on_wait=[w], on_update=[])
                    out.append(nop)
                inst.sync_info = mybir.SyncInfo(
                    on_wait=[waits[-1]], on_update=list(si.on_update)
                )
                changed = True
            out.append(inst)
        if changed:
            insts[:] = out


def kernel(input, weight, tokens_per_expert):
    global LAST_EXEC_TIME_NS, LAST_TRACE, LAST_RESULT
    from concourse.bass_utils import run_bass_kernel_spmd

    x = np.asarray(input, dtype=np.float32)
    w = np.asarray(weight, dtype=np.float32)
    sizes = tuple(int(s) for s in np.asarray(tokens_per_expert).reshape(-1))
    assert sum(sizes) == NUM_TOKENS and len(sizes) == GROUPS
    assert x.shape == (NUM_TOKENS, IN_FEATURES)
    assert w.shape == (GROUPS, IN_FEATURES, OUT_FEATURES)

    dt_name = os.environ.get("GG_DTYPE", "bfloat16")
    out_dt_name = os.environ.get("GG_OUT_DTYPE", "bfloat16")
    import ml_dtypes

    np_dt = {"bfloat16": ml_dtypes.bfloat16, "float32": np.float32}[dt_name]

    reps = int(os.environ.get("GG_REPS", "1"))
    key = (sizes, dt_name, out_dt_name, reps)
    if key not in _COMPILED:
        _COMPILED[key] = _build(sizes, dt_name, out_dt_name, reps)
    nc = _COMPILED[key]

    order, waves_after, offs, chunks, tails, x_total = _layout(sizes)

    # packed x: per chunk (and per tail), all 20 k-tiles contiguous on
    # each partition line -> single-descriptor DMAs
    xp0 = (
        x.T.reshape(K_TILES, P, NUM_TOKENS).transpose(1, 0, 2).astype(np_dt)
    )  # [P, K, T]
    xbuf = np.empty((P, x_total), dtype=np_dt)
    for g in order:
        a = offs[g]
        for fo, cbase, clen in chunks[g]:
            xbuf[:, fo : fo + K_TILES * clen] = xp0[
                :, :, a + cbase : a + cbase + clen
            ].reshape(P, -1)
        if g in tails:
            fo, tb, tl = tails[g]
            xbuf[:, fo : fo + K_TILES * tl] = xp0[:, :, a + tb : a + tb + tl].reshape(
                P, -1
            )

    in_maps = []
    for c in range(N_CORES):
        # w_packed [G, P, K, C]: line (g,p) holds k-major, col-minor runs
        wc = np.ascontiguousarray(
            w[:, :, c * COLS : (c + 1) * COLS]
            .reshape(GROUPS, K_TILES, P, COLS)
            .transpose(0, 2, 1, 3)
        ).astype(np_dt)
        in_maps.append(
            {
                "xt": xbuf,
                "wt": wc.reshape(GROUPS, P, K_TILES * COLS),
            }
        )

    trace = os.environ.get("GG_TRACE", "0") == "1"
    res = run_bass_kernel_spmd(nc, in_maps, list(range(N_CORES)), trace=trace)
    LAST_EXEC_TIME_NS = res.exec_time_ns
    LAST_RESULT = res
    if res.instructions_and_trace is not None:
        LAST_TRACE = res.instructions_and_trace[1]

    cores = []
    for c in range(N_CORES):
        oc = np.asarray(res.results[c]["out"]).copy()
        tc = np.asarray(res.results[c]["tout"])
        for wi, j, g, soff, mt in nc._gg_wave_map:
            oc[offs[g] + soff : offs[g] + soff + mt, :] = tc[
                wi * P + SUB * j : wi * P + SUB * j + mt, :
            ]
        cores.append(oc)
    out = np.concatenate(cores, axis=1).astype(np.float32)
    return out


# revision 40
# speedup vs baseline: 1.0875x; 1.0875x over previous
# Grouped GEMM (MoE) kernel for Trainium2, 8 NeuronCores.
#
# Sharding: tensor-parallel over out_features (column parallel). Each core
# computes ALL 4096 tokens against its own 416-column slice of every
# expert's weight. No collectives; host concatenates per-core outputs
# along the feature axis. This is perfectly load balanced across cores
# regardless of the (uneven) per-expert token counts, and the program is
# identical on every core (SPMD) -- only the weight *values* differ.
#
# Dtype: inputs are cast to bf16 on host (PE runs bf16 at 4x the fp32
# rate; tolerance 2e-2 vs ~2.5e-3 bf16 error). Output is stored bf16 and
# upcast on host, halving store traffic.
#
# The kernel is INPUT-DMA-BOUND: ~42MB/core over 16 DMA engines that
# peak ~22GB/s each. Measured engine efficiency is ~84% with the naive
# layout because HWDGE emits one descriptor per (partition, k-tile) line
# (832-2048B) and pays ~7ns per descriptor. So the host packs every
# transfer as ONE contiguous per-partition run (8-20KB descriptors):
#   x: per 512-token chunk, all 20 k-tiles contiguous [p | k | t]
#   w: per 10-k-tile half slice, [p | k | c] (already contiguous)
#   x tails (for the tail waves): packed separately the same way
# SBUF tiles are allocated FLAT and rearranged into [p, k, *] views for
# the matmuls, so the DMA sees identical src/dst patterns.
#
# PE structure:
#  * psum[tok, col] += x_tile[k, tok].T @ w_tile[k, col], k accumulated
#    over 20 k-tiles per 128-token m-tile.
#  * Partial (tail) m-tiles are NOT processed as full 416-cycle m-tiles.
#    Tails are split into <=32-token subtiles and packed 4-at-a-time into
#    the PE's 32-column groups via tile_position=(0,32j): the 4 streams
#    run concurrently, so a wave of 4 tails costs ~1 m-tile instead of 4.
#  * HAM warm-up: ~12 dummy matmuls on a memset tile run during the DMA
#    ramp so the PE clock gate is at 8/8 when the real matmuls start.
#  * Expert order is searched (all permutations, simulated) to keep the
#    cumulative DMA demand behind the cumulative PE supply.
#
# All input DMAs issue from SP in w-half0, chunk0, w-half1, chunk1 order
# per expert: the DMA engines serve queues at similar rates regardless
# of backlog, so splitting w/x across queues skews their delivery ratio
# (w on GpSimd's queue measured ~10us slower; x on ACT 41us slower).
# Stores issue from ACT so their waits never stall input issue; the tiny
# tail-x loads issue from GpSimd.

import os

import numpy as np

NUM_TOKENS = 4096
IN_FEATURES = 2560
OUT_FEATURES = 3328
GROUPS = 8
N_CORES = 8
COLS = OUT_FEATURES // N_CORES  # 416
P = 128
K_TILES = IN_FEATURES // P  # 20
K_PIECES = (5, 5, 10)  # w pieces: the small leading pieces unblock the
# first matmuls ~2us earlier during the ramp (0.53MB vs 1.06MB wait)
CHUNK = 512  # tokens per x chunk; chunks release their pool slot at
# their own last m-tile (not expert end) so the 6-deep rotation gives
# ~3 experts of DMA lookahead and SP's in-order issue rarely blocks
SUB = 32  # tail subtile height (PE column-group width)
W_BUFS = 4  # weight pool depth (experts in flight per k-half tag)
X_BUFS = 5  # x pool depth (512-token chunks in flight); one unit of
# depth is traded for a dedicated last-expert chunk buffer (see below)

LAST_EXEC_TIME_NS = None
LAST_TRACE = None
LAST_RESULT = None

_COMPILED = {}
_LAYOUTS = {}


def _plan(sizes):
    """Pick expert order + tail-wave packing.

    A tail wave is <=4 subtiles (each <=32 tokens) matmul'd concurrently
    in the PE's four 32-column groups. A wave reads the w and tail-x SBUF
    tiles of its member experts, so it must be emitted before those
    pool slots are reallocated (w rotates W_BUFS deep, tail-x 3 deep)
    => a member of age a (experts processed since it) needs a <= 2.

    Search all expert permutations; minimize wave count, prefer a
    tail-less final expert, then maximize the worst prefix slack of
    (PE work supplied) - (DMA bytes demanded).
    """
    import itertools

    max_age = 2
    nz = [g for g in range(GROUPS) if int(sizes[g]) > 0]
    full = {g: int(sizes[g]) // P for g in nz}
    subs = {}
    for g in nz:
        t = int(sizes[g]) % P
        s, off = [], full[g] * P
        while t > 0:
            m = min(SUB, t)
            s.append((off, m))
            off += m
            t -= m
        subs[g] = s

    mm_us = K_TILES * COLS / 2.4e3  # warm full-tile cost, us
    wave_us = K_TILES * (COLS / 2.4 + 12) / 1e3
    bw = 0.35  # GB/s -> MB/us
    wslice_mb = IN_FEATURES * COLS * 2 / 1e6

    def simulate(order):
        pending = []  # (pos, g, off, mt)
        waves_after = [[] for _ in order]
        pe = dma = 0.0
        min_slack = 1e9
        nwaves = 0
        for pos, g in enumerate(order):
            dma += wslice_mb + int(sizes[g]) * IN_FEATURES * 2 / 1e6
            pe += full[g] * mm_us
            pending += [(pos, g, o, m) for (o, m) in subs[g]]
            last = pos == len(order) - 1
            while pending and (
                last or pos - pending[0][0] >= max_age or len(pending) > 4
            ):
                wave = pending[:4]
                pending = pending[4:]
                waves_after[pos].append([(g2, o, m) for (_, g2, o, m) in wave])
                nwaves += 1
                pe += wave_us
            min_slack = min(min_slack, pe - dma / bw)
        return nwaves, min_slack, waves_after

    best = None
    for order in itertools.permutations(nz):
        nwaves, slack, waves_after = simulate(order)
        ends_with_tail = 1 if subs[order[-1]] else 0
        key = (nwaves, ends_with_tail, -slack)
        if best is None or key < best[0]:
            best = (key, order, waves_after)
    return best[1], best[2]


def _layout(sizes):
    """Flat element layout of the packed x buffer (per partition line).

    chunks[g] = [(flat_off, cbase, clen), ...]
    tails[g] = (flat_off, tail_base, tail_len)
    """
    if sizes in _LAYOUTS:
        return _LAYOUTS[sizes]
    order, waves_after = _plan(sizes)
    offs = [0]
    for s in sizes:
        offs.append(offs[-1] + int(s))
    pos = 0
    chunks = {}
    tails = {}
    for g in order:
        seg = int(sizes[g])
        li = []
        # first expert gets a one-m-tile starter chunk so the first real
        # matmul only waits for 1.7MB (w-half0 + 128 tokens) of DMA
        # instead of 3.7MB -- cuts ~5us off the pipeline ramp
        # the first expert's chunks grow progressively [128,128,256,512..]
        # so the DMA supply curve tracks the PE's m-tile demand curve
        # during the pipeline ramp instead of stalling m-tiles 1-4 behind
        # one 2.6MB chunk
        cbase = 0
        if g == order[0]:
            for clen in (P, P, 2 * P):
                if cbase + clen > seg:
                    break
                li.append((pos, cbase, clen))
                pos += K_TILES * clen
                cbase += clen
        while cbase < seg:
            clen = min(CHUNK, seg - cbase)
            li.append((pos, cbase, clen))
            pos += K_TILES * clen
            cbase += clen
        chunks[g] = li
        tl = seg % P
        if tl:
            tails[g] = (pos, (seg // P) * P, tl)
            pos += K_TILES * tl
    _LAYOUTS[sizes] = (order, waves_after, offs, chunks, tails, pos)
    return _LAYOUTS[sizes]


def _build(sizes, dt_name, out_dt_name, reps=1):
    import concourse.bass as bass
    import concourse.mybir as mybir
    import concourse.tile as tile

    dt_in = getattr(mybir.dt, dt_name)
    dt_out = getattr(mybir.dt, out_dt_name)
    f32 = mybir.dt.float32

    order, waves_after, offs, chunks, tails, x_total = _layout(sizes)

    nc = bass.Bass()
    xt_d = nc.dram_tensor("xt", [P, x_total], dt_in, kind="ExternalInput")
    wt_d = nc.dram_tensor(
        "wt", [GROUPS, P, K_TILES * COLS], dt_in, kind="ExternalInput"
    )
    out_d = nc.dram_tensor("out", [NUM_TOKENS, COLS], dt_out, kind="ExternalOutput")
    # tail-wave staging: one [128, COLS] block per wave, scattered to the
    # right output rows on the host (one big store beats 4 tiny ones)
    n_waves_max = (GROUPS * 4 + 3) // 4
    tout_d = nc.dram_tensor(
        "tout", [n_waves_max * P, COLS], dt_out, kind="ExternalOutput"
    )

    # Tile's default kernel tail is: drain -> barrier -> clear all tile
    # semaphores -> barrier. The drain already guarantees every DMA
    # completed; the sem clears only matter for re-executing the same
    # loaded NEFF, which the runtime re-inits anyway. Keep drain + one
    # barrier, skip the clears.
    from concourse.vector_clock import ScopedClock

    if not hasattr(tile.TileContext, "_orig_drain_and_barrier"):
        tile.TileContext._orig_drain_and_barrier = tile.TileContext._drain_and_barrier

    def _short_drain_and_barrier(self, tick_clock, wait_clock):
        if os.environ.get("GG_FULL_TAIL", "0") == "1":
            return tile.TileContext._orig_drain_and_barrier(
                self, tick_clock, wait_clock
            )
        drain_inst = self.nc.sync.drain()
        wait_clock.add_sem_waits(
            drain_inst.ins, ScopedClock({None: tick_clock.global_clock})
        )
        self.nc.all_engine_barrier()
        popped = self.nc._tile_sem_poison_stack.pop()
        assert popped is self._sem_poison

    tile.TileContext._drain_and_barrier = _short_drain_and_barrier

    with tile.TileContext(nc) as tc:
        with (
            tc.tile_pool(name="wp", bufs=W_BUFS) as wp,
            tc.tile_pool(name="xp", bufs=X_BUFS) as xp,
            tc.tile_pool(name="xtp", bufs=3) as xtp,
            tc.tile_pool(name="pp", bufs=6, space="PSUM") as pp,
            tc.tile_pool(name="op", bufs=3) as op,
        ):
            def body():
                _emit_body(
                    nc, wp, xp, xtp, pp, op, sizes, dt_in, dt_out, f32,
                    xt_d, wt_d, out_d, tout_d,
                    order, waves_after, offs, chunks, tails,
                )

            if reps > 1:
                with tc.For_i(0, reps, 1):
                    body()
            else:
                body()

    _split_waits(nc, mybir)
    nc.finalize()
    return nc


def _emit_body(nc, wp, xp, xtp, pp, op, sizes, dt_in, dt_out, f32,
               xt_d, wt_d, out_d, tout_d, order, waves_after, offs, chunks, tails):
    tw = (max(1, max(int(s) % P for s in sizes)) + 7) // 8 * 8

    # --- HAM warm-up: ~6us of dummy matmuls with no DMA deps. They run
    # during the DMA ramp (PE would be idle anyway) and flip the PE clock
    # gate to 8/8 before the first real matmul arrives.
    wrm = op.tile([P, SUB + COLS], dt_in, tag="wrm", bufs=1, name="wrm")
    nc.vector.memset(wrm[:, :], 0)
    for i in range(16):
        wps = pp.tile([P, COLS], f32, tag="wv", bufs=2, name=f"wps_{i}")
        nc.tensor.matmul(
            wps[:SUB, :P],
            wrm[:, :SUB],
            wrm[:, SUB : SUB + P],
            start=True,
            stop=True,
        )

    kb = [0]
    for n in K_PIECES:
        kb.append(kb[-1] + n)  # piece q covers k in [kb[q], kb[q+1])

    wtiles = {}  # g -> [k-view per half]
    xtiles = {}  # g -> [k-view per chunk]
    xtails = {}  # g -> (k-view, tail_base)

    wave_idx = [0]
    wave_map = []  # (wave_slot, j, g, soff, mt) for the host scatter

    def emit_wave(wave):
        ps = pp.tile([P, COLS], f32, tag="wv", bufs=2, name="ps_wave")
        for k in range(K_TILES):
            q = next(i for i in range(len(K_PIECES)) if kb[i + 1] > k)
            r = k - kb[q]
            for j, (g, soff, mt) in enumerate(wave):
                xtv, tb = xtails[g]
                sc = soff - tb
                nc.tensor.matmul(
                    ps[SUB * j : SUB * j + mt, :],
                    xtv[:, k, sc : sc + mt],
                    wtiles[g][q][:, r, :],
                    start=(k == 0),
                    stop=(k == K_TILES - 1),
                    tile_position=(0, SUB * j),
                )
        ob = op.tile([P, COLS], dt_out, tag="o", name="ob_wave")
        nc.vector.tensor_copy(ob[:, :], ps[:, :])
        wi = wave_idx[0]
        wave_idx[0] += 1
        nc.scalar.dma_start(tout_d[wi * P : (wi + 1) * P, :], ob[:, :])
        for j, (g, soff, mt) in enumerate(wave):
            wave_map.append((wi, j, g, soff, mt))

    for pos, g in enumerate(order):
        seg = int(sizes[g])
        off = offs[g]
        wtiles[g] = []
        if g not in xtiles:
            xtiles[g] = []

        def emit_x(ci, tag="x"):
            fo, cbase, clen = chunks[g][ci]
            xt = xp.tile([P, K_TILES * clen], dt_in, tag=tag, name=f"x_{g}_{ci}")
            nc.sync.dma_start(xt[:, :], xt_d[:, fo : fo + K_TILES * clen])
            xtiles[g].append(xt[:, :].rearrange("p (k t) -> p k t", k=K_TILES))

        def emit_w(q):
            kn = K_PIECES[q]
            wt = wp.tile([P, kn * COLS], dt_in, tag=f"w{q}", name=f"w_{g}_{q}")
            nc.sync.dma_start(
                wt[:, :], wt_d[g, :, kb[q] * COLS : kb[q + 1] * COLS]
            )
            wtiles[g].append(wt[:, :].rearrange("p (k c) -> p k c", k=kn))

        already_loaded = bool(xtiles[g])
        emit_w(0)
        if not already_loaded:
            emit_x(0)
        for q in range(1, len(K_PIECES)):
            emit_w(q)
        if not already_loaded:
            for ci in range(1, len(chunks[g])):
                emit_x(ci)
        # the LAST expert's x chunk is otherwise the final input DMA and
        # lands right at the PE's convergence point (~4us starve). Load it
        # into a dedicated buffer right behind the first expert's data.
        if pos == 0 and len(order) > 1 and len(chunks[order[-1]]) == 1:
            lg = order[-1]
            lfo, _, lclen = chunks[lg][0]
            xlt = xp.tile(
                [P, K_TILES * lclen], dt_in, tag="xl", bufs=1, name=f"x_{lg}_early"
            )
            nc.sync.dma_start(xlt[:, :], xt_d[:, lfo : lfo + K_TILES * lclen])
            xtiles[lg] = [xlt[:, :].rearrange("p (k t) -> p k t", k=K_TILES)]
        if g in tails:
            fo, tb, tl = tails[g]
            xtt = xtp.tile([P, K_TILES * tw], dt_in, tag="xt", name=f"xtail_{g}")
            nc.gpsimd.dma_start(
                xtt[:, : K_TILES * tl], xt_d[:, fo : fo + K_TILES * tl]
            )
            xtails[g] = (
                xtt[:, : K_TILES * tl].rearrange("p (k t) -> p k t", k=K_TILES),
                tb,
            )

        # on the last expert, flush the remaining waves BEFORE its m-loop:
        # their stores then overlap its compute instead of serializing at
        # the very end of the kernel (members are from recent experts, so
        # their tiles are still live)
        if pos == len(order) - 1:
            for wave in waves_after[pos]:
                emit_wave(wave)

        # full 128-token m-tiles only; tails go to the packed waves
        for ci in range(len(chunks[g])):
            fo, cbase, clen = chunks[g][ci]
            n_full = clen // P
            for m in range(n_full):
                ps = pp.tile([P, COLS], f32, tag="ps", name=f"ps_{g}_{ci}_{m}")
                for k in range(K_TILES):
                    q = next(i for i in range(len(K_PIECES)) if kb[i + 1] > k)
                    r = k - kb[q]
                    nc.tensor.matmul(
                        ps[:, :],
                        xtiles[g][ci][:, k, m * P : (m + 1) * P],
                        wtiles[g][q][:, r, :],
                        start=(k == 0),
                        stop=(k == K_TILES - 1),
                    )
                ob = op.tile([P, COLS], dt_out, tag="o", name=f"ob_{g}_{ci}_{m}")
                nc.vector.tensor_copy(ob[:, :], ps[:, :])
                r0 = off + cbase + m * P
                nc.scalar.dma_start(out_d[r0 : r0 + P, :], ob[:, :])

        if pos < len(order) - 1:
            for wave in waves_after[pos]:
                emit_wave(wave)

    nc._gg_wave_map = wave_map


def _split_waits(nc, mybir):
    """This container's walrus build allows at most ONE sync wait per
    instruction ('Too many sync wait commands' otherwise). Split any
    instruction carrying N>1 waits into N-1 same-engine NoOps (one wait
    each) followed by the original instruction with the last wait. Engine
    sequencers execute in order, so semantics are preserved."""
    counter = [0]
    for blk in nc.m.functions[0].blocks:
        insts = blk.instructions
        out = []
        changed = False
        for inst in insts:
            si = inst.sync_info
            if si is not None and len(si.on_wait) > 1:
                waits = list(si.on_wait)
                for w in waits[:-1]:
                    counter[0] += 1
                    nop = mybir.InstNoOp(name=f"I-nopw-{counter[0]}")
                    nop.engine = inst.engine
                    nop.sync_info = mybir.SyncInfo(on_wait=[w], on_update=[])
                    out.append(nop)
                inst.sync_info = mybir.SyncInfo(
                    on_wait=[waits[-1]], on_update=list(si.on_update)
                )
                changed = True
            out.append(inst)
        if changed:
            insts[:] = out


def kernel(input, weight, tokens_per_expert):
    global LAST_EXEC_TIME_NS, LAST_TRACE, LAST_RESULT
    from concourse.bass_utils import run_bass_kernel_spmd

    x = np.asarray(input, dtype=np.float32)
    w = np.asarray(weight, dtype=np.float32)
    sizes = tuple(int(s) for s in np.asarray(tokens_per_expert).reshape(-1))
    assert sum(sizes) == NUM_TOKENS and len(sizes) == GROUPS
    assert x.shape == (NUM_TOKENS, IN_FEATURES)
    assert w.shape == (GROUPS, IN_FEATURES, OUT_FEATURES)

    dt_name = os.environ.get("GG_DTYPE", "bfloat16")
    out_dt_name = os.environ.get("GG_OUT_DTYPE", "bfloat16")
    import ml_dtypes

    np_dt = {"bfloat16": ml_dtypes.bfloat16, "float32": np.float32}[dt_name]

    reps = int(os.environ.get("GG_REPS", "1"))
    key = (sizes, dt_name, out_dt_name, reps)
    if key not in _COMPILED:
        _COMPILED[key] = _build(sizes, dt_name, out_dt_name, reps)
    nc = _COMPILED[key]

    order, waves_after, offs, chunks, tails, x_total = _layout(sizes)

    # packed x: per chunk (and per tail), all 20 k-tiles contiguous on
    # each partition line -> single-descriptor DMAs
    xp0 = (
        x.T.reshape(K_TILES, P, NUM_TOKENS).transpose(1, 0, 2).astype(np_dt)
    )  # [P, K, T]
    xbuf = np.empty((P, x_total), dtype=np_dt)
    for g in order:
        a = offs[g]
        for fo, cbase, clen in chunks[g]:
            xbuf[:, fo : fo + K_TILES * clen] = xp0[
                :, :, a + cbase : a + cbase + clen
            ].reshape(P, -1)
        if g in tails:
            fo, tb, tl = tails[g]
            xbuf[:, fo : fo + K_TILES * tl] = xp0[:, :, a + tb : a + tb + tl].reshape(
                P, -1
            )

    in_maps = []
    for c in range(N_CORES):
        # w_packed [G, P, K, C]: line (g,p) holds k-major, col-minor runs
        wc = np.ascontiguousarray(
            w[:, :, c * COLS : (c + 1) * COLS]
            .reshape(GROUPS, K_TILES, P, COLS)
            .transpose(0, 2, 1, 3)
        ).astype(np_dt)
        in_maps.append(
            {
                "xt": xbuf,
                "wt": wc.reshape(GROUPS, P, K_TILES * COLS),
            }
        )

    trace = os.environ.get("GG_TRACE", "0") == "1"
    res = run_bass_kernel_spmd(nc, in_maps, list(range(N_CORES)), trace=trace)
    LAST_EXEC_TIME_NS = res.exec_time_ns
    LAST_RESULT = res
    if res.instructions_and_trace is not None:
        LAST_TRACE = res.instructions_and_trace[1]

    cores = []
    for c in range(N_CORES):
        oc = np.asarray(res.results[c]["out"]).copy()
        tc = np.asarray(res.results[c]["tout"])
        for wi, j, g, soff, mt in nc._gg_wave_map:
            oc[offs[g] + soff : offs[g] + soff + mt, :] = tc[
                wi * P + SUB * j : wi * P + SUB * j + mt, :
            ]
        cores.append(oc)
    out = np.concatenate(cores, axis=1).astype(np.float32)
    return out
